# revision 22
# baseline (speedup 1.0000x reference)
import os
import sys

sys.path.insert(0, "/opt/trn_rl_repo")
os.environ.setdefault("MYCRO_LOCAL_CACHE", "1")
os.environ.setdefault("NEURON_RT_RESET_CORES", "1")

import numpy as np

N_CORES = 8
P = 128  # partition / tile size
HP = 64  # paired rows per tile

last_exec_time_ns = None


def _preprocess(rows, cols, vals, per_core, n_tiles, group_tiles, bank, nb):
    """Lay out edges into SPMD-uniform slot arrays for dma_gather + segsum.

    Source features are stored PAIRED: nodes n and n+64 of the same tile
    share paired row r = (n//128)*64 + (n%64); n's features occupy half
    h = (n%128)//64 of the 2*F-wide row.  One 512B gather descriptor
    fetches both nodes' features; the scatter-matmul uses two half-masked
    selection matrices (val_h0/val_h1) per slot column.

    Block columns are ordered: group g -> source bank b -> tile t-in-group.
    Edge at slot (partition p, column j) has paired source row (bank-
    relative int16 at flat position j*128+p of idx16), weights
    val_h0/val_h1[p,j] (exactly one nonzero unless pad), and dest row
    dst[p,j] (0..127) within its dest tile.  Pad slots: idx=0, vals=0.
    """
    E = rows.shape[0]
    n_groups = -(-n_tiles // group_tiles)
    core = rows // per_core
    loc = rows - core * per_core
    t_loc = loc // P
    d_loc = loc - t_loc * P
    g_loc = t_loc // group_tiles
    t_in_g = t_loc - g_loc * group_tiles

    Tg = cols // P
    w = cols - Tg * P
    r_pair = Tg * HP + (w % HP)
    half = w // HP
    b_src = r_pair // bank
    h2 = r_pair - b_src * bank

    key = ((core * n_groups + g_loc) * nb + b_src) * group_tiles + t_in_g
    nkeys = N_CORES * n_groups * nb * group_tiles
    order = np.argsort(key, kind="stable")
    counts = np.bincount(key, minlength=nkeys)
    cnt4 = counts.reshape(N_CORES, n_groups, nb, group_tiles)
    K = (cnt4.max(axis=0) + P - 1) // P  # [n_groups, nb, group_tiles]
    for g in range(n_groups):
        for ti in range(group_tiles):
            t = g * group_tiles + ti
            if t >= n_tiles:
                K[g, :, ti] = 0
            elif K[g, :, ti].sum() == 0:
                K[g, 0, ti] = 1

    col_of = np.zeros((n_groups, nb, group_tiles), dtype=np.int64)
    grp_base = np.zeros(n_groups + 1, dtype=np.int64)
    seg = np.zeros((n_groups, nb, 2), dtype=np.int64)  # (start, len)
    run = 0
    for g in range(n_groups):
        grp_base[g] = run
        for b in range(nb):
            seg[g, b, 0] = run
            for ti in range(group_tiles):
                col_of[g, b, ti] = run
                run += int(K[g, b, ti])
            seg[g, b, 1] = run - seg[g, b, 0]
    grp_base[n_groups] = run
    TOT = int(run)

    key_s = key[order]
    grp_start = np.concatenate([[0], np.cumsum(counts)])
    ranks = np.arange(E, dtype=np.int64) - grp_start[key_s]
    cs = core[order]
    gs = g_loc[order]
    bs = b_src[order]
    tis = t_in_g[order]
    p = ranks % P
    colj = col_of[gs, bs, tis] + ranks // P

    idx_flat = np.zeros((N_CORES, TOT * P), dtype=np.int16)
    idx_flat[cs, colj * P + p] = h2[order].astype(np.int16)
    val_a = np.zeros((N_CORES, P, TOT), dtype=np.float32)
    dst_a = np.zeros((N_CORES, P, TOT), dtype=np.float32)
    val_a[cs, p, colj] = vals[order]
    # dst2 encodes (dest row, pair half): iota256 == dst + 128*half
    dst_a[cs, p, colj] = (d_loc[order] + P * half[order]).astype(np.float32)

    # dma_gather index layout: flat i at partition i%16, col i//16,
    # replicated across the 8 groups of 16 partitions.
    idx16 = np.empty((N_CORES, P, TOT * 8), dtype=np.int16)
    for c in range(N_CORES):
        a = idx_flat[c].reshape(-1, 16).T
        idx16[c] = np.tile(a, (8, 1))

    tile_cols = []
    for t in range(n_tiles):
        g, ti = t // group_tiles, t % group_tiles
        cl = []
        for b in range(nb):
            c0 = int(col_of[g, b, ti])
            cl.extend(range(c0, c0 + int(K[g, b, ti])))
        tile_cols.append(cl)

    layout = dict(n_groups=n_groups, TOT=TOT, grp_base=grp_base, seg=seg,
                  tile_cols=tile_cols, K=K, idx_flat=idx_flat)
    return idx16, val_a, dst_a, layout



def _preprocess2(rows, cols, vals, per_core, n_tiles, group_tiles):
    """Layer-2 slot layout: y2 stored QUAD-packed, 4 nodes per 512B row.
    Node n -> quad row r4 = (n//128)*32 + (n%32), quarter (n%128)//32.
    One bank (NP/4 <= 32767 rows); dst3 encodes dest row + 128*quarter
    (0..511), compared against an fp16 iota512."""
    E = rows.shape[0]
    QP = 32
    n_groups = -(-n_tiles // group_tiles)
    core = rows // per_core
    loc = rows - core * per_core
    t_loc = loc // P
    d_loc = loc - t_loc * P
    g_loc = t_loc // group_tiles
    t_in_g = t_loc - g_loc * group_tiles

    Tg = cols // P
    w = cols - Tg * P
    r4 = Tg * QP + (w % QP)
    quarter = w // QP

    key = (core * n_groups + g_loc) * group_tiles + t_in_g
    nkeys = N_CORES * n_groups * group_tiles
    order = np.argsort(key, kind="stable")
    counts = np.bincount(key, minlength=nkeys)
    cnt3 = counts.reshape(N_CORES, n_groups, group_tiles)
    K = (cnt3.max(axis=0) + P - 1) // P  # [n_groups, group_tiles]
    for g in range(n_groups):
        for ti in range(group_tiles):
            t = g * group_tiles + ti
            if t >= n_tiles:
                K[g, ti] = 0
            elif K[g, ti] == 0:
                K[g, ti] = 1

    col_of = np.zeros((n_groups, group_tiles), dtype=np.int64)
    grp_base = np.zeros(n_groups + 1, dtype=np.int64)
    run = 0
    for g in range(n_groups):
        grp_base[g] = run
        for ti in range(group_tiles):
            col_of[g, ti] = run
            run += int(K[g, ti])
    grp_base[n_groups] = run
    TOT = int(run)

    key_s = key[order]
    grp_start = np.concatenate([[0], np.cumsum(counts)])
    ranks = np.arange(E, dtype=np.int64) - grp_start[key_s]
    cs = core[order]
    gs = g_loc[order]
    tis = t_in_g[order]
    p = ranks % P
    colj = col_of[gs, tis] + ranks // P

    idx_flat = np.zeros((N_CORES, TOT * P), dtype=np.int16)
    idx_flat[cs, colj * P + p] = r4[order].astype(np.int16)
    val_a = np.zeros((N_CORES, P, TOT), dtype=np.float32)
    dst_a = np.zeros((N_CORES, P, TOT), dtype=np.float32)
    val_a[cs, p, colj] = vals[order]
    dst_a[cs, p, colj] = (d_loc[order] + P * quarter[order]).astype(
        np.float32)

    idx16 = np.empty((N_CORES, P, TOT * 8), dtype=np.int16)
    for c in range(N_CORES):
        a = idx_flat[c].reshape(-1, 16).T
        idx16[c] = np.tile(a, (8, 1))

    tile_cols = []
    for t in range(n_tiles):
        g, ti = t // group_tiles, t % group_tiles
        c0 = int(col_of[g, ti])
        tile_cols.append(list(range(c0, c0 + int(K[g, ti]))))

    layout = dict(n_groups=n_groups, TOT=TOT, grp_base=grp_base,
                  tile_cols=tile_cols, K=K, idx_flat=idx_flat)
    return idx16, val_a, dst_a, layout


def _build_program(F1, F2, per_core, n_tiles, group_tiles, bank, nb, layout,
                   layout2):
    import concourse.bass as bass
    import concourse.bacc as bacc
    import concourse.mybir as mybir
    import concourse.tile as tile

    fp32 = mybir.dt.float32
    bf16 = mybir.dt.bfloat16
    i16 = mybir.dt.int16
    NP_ = per_core * N_CORES
    PR = NP_ // 2  # paired rows
    pr_core = per_core // 2
    AF = mybir.ActivationFunctionType
    OP = mybir.AluOpType

    TOT = layout["TOT"]
    grp_base = layout["grp_base"]
    seg = layout["seg"]
    tile_cols = layout["tile_cols"]
    n_groups = layout["n_groups"]
    TOT2 = layout2["TOT"]
    grp_base2 = layout2["grp_base"]
    tile_cols2 = layout2["tile_cols"]
    QP = P // 4  # quad rows per tile
    QR = per_core * N_CORES // 4  # total quad rows

    nc = bacc.Bacc("TRN2", target_bir_lowering=False, debug=False,
                   num_devices=N_CORES)
    xT_ext = nc.dram_tensor("xT", [F1, per_core], bf16, kind="ExternalInput")
    w1_ext = nc.dram_tensor("w1", [F1, F1], bf16, kind="ExternalInput")
    b1_ext = nc.dram_tensor("b1", [1, F1], bf16, kind="ExternalInput")
    w2_ext = nc.dram_tensor("w2", [F1, F2], fp32, kind="ExternalInput")
    b2_ext = nc.dram_tensor("b2", [1, F2], fp32, kind="ExternalInput")
    idx16_ext = nc.dram_tensor("idx16", [P, TOT * 8], i16,
                               kind="ExternalInput")
    val_ext = nc.dram_tensor("val", [P, TOT], fp32, kind="ExternalInput")
    dst_ext = nc.dram_tensor("dst", [P, TOT], fp32, kind="ExternalInput")
    iotab_ext = nc.dram_tensor("iotab", [P, 2 * P], bf16,
                               kind="ExternalInput")
    idx2_ext = nc.dram_tensor("idx2", [P, TOT2 * 8], i16,
                              kind="ExternalInput")
    val2_ext = nc.dram_tensor("val2", [P, TOT2], fp32, kind="ExternalInput")
    dst3_ext = nc.dram_tensor("dst3", [P, TOT2], fp32, kind="ExternalInput")
    iota512_ext = nc.dram_tensor("iota512", [P, 4 * P], mybir.dt.float16,
                                 kind="ExternalInput")
    ident_ext = nc.dram_tensor("ident", [P, P], fp32, kind="ExternalInput")
    out_ext = nc.dram_tensor("out", [per_core, F2], fp32,
                             kind="ExternalOutput")

    with tile.TileContext(nc) as tc:
        with tc.tile_pool(name="static", bufs=1) as static, \
             tc.tile_pool(name="dram", bufs=1, space="DRAM") as dram:
            w1_sb = static.tile([F1, F1], bf16)
            nc.sync.dma_start(w1_sb[:], w1_ext[:])
            b1_sb = static.tile([1, F1], bf16)
            nc.sync.dma_start(b1_sb[:], b1_ext[:])
            w2_sb = static.tile([F1, F2], fp32)
            nc.sync.dma_start(w2_sb[:], w2_ext[:])
            b2_sb = static.tile([1, F2], fp32)
            nc.sync.dma_start(b2_sb[:], b2_ext[:])
            idx16_sb = static.tile([P, TOT * 8], i16)
            val_sb = static.tile([P, TOT], fp32)
            dst_sb = static.tile([P, TOT], fp32)
            idx2_sb = static.tile([P, TOT2 * 8], i16)
            val2_sb = static.tile([P, TOT2], fp32)
            dst3_sb = static.tile([P, TOT2], fp32)
            iota512 = static.tile([P, 4 * P], mybir.dt.float16)

            ones_sb = static.tile([1, P], fp32)
            nc.vector.memset(ones_sb[:], 1.0)
            ones_b = static.tile([1, P], bf16)
            nc.vector.memset(ones_b[:], 1.0)
            # iota / identity come from host: avoids InstIota (library 0)
            # which deadlocks on HW when interleaved with dma_gather (lib 3).
            iotab = static.tile([P, 2 * P], bf16)
            nc.sync.dma_start(iotab[:], iotab_ext[:])
            ident = static.tile([P, P], fp32)
            nc.sync.dma_start(ident[:], ident_ext[:])

            hp_dram = dram.tile([pr_core, 2 * F1], bf16)
            HP_full = dram.tile([PR, 2 * F1], bf16, addr_space="Shared",
                                name="HP_full")
            y2q_dram = dram.tile([per_core // 4, 4 * F2], bf16)
            Y2Q_full = dram.tile([QR, 4 * F2], bf16, addr_space="Shared",
                                 name="Y2Q_full")

            # ---- Phase A: h = x @ W1 + b1  (own nodes; xT from host) ----
            XB = 14  # tiles per DMA batch (98 = 7*14)
            assert n_tiles % XB == 0
            with tc.tile_pool(name="xa", bufs=2) as xa, \
                 tc.tile_pool(name="ha", bufs=2) as ha, \
                 tc.tile_pool(name="psB", bufs=4,
                              space=bass.MemorySpace.PSUM) as psB:
                for t0 in range(0, n_tiles, XB):
                    xT_sb = xa.tile([F1, XB, P], bf16)
                    nc.sync.dma_start(
                        xT_sb[:],
                        xT_ext[:, t0 * P:(t0 + XB) * P]
                        .rearrange("f (a p) -> f a p", p=P))
                    h_sb = ha.tile([P, XB, F1], bf16)
                    for i in range(XB):
                        h_ps = psB.tile([P, F1], fp32)
                        nc.tensor.matmul(h_ps[:], ones_b[:], b1_sb[:],
                                         start=True, stop=False)
                        nc.tensor.matmul(h_ps[:], xT_sb[:, i, :], w1_sb[:],
                                         start=False, stop=True)
                        nc.scalar.activation(h_sb[:, i, :], h_ps[:], AF.Copy)
                    # paired store: node (t0+a)*128+p -> row (t0+a)*64+p%64,
                    # half p//64
                    nc.scalar.dma_start(
                        hp_dram[t0 * HP:(t0 + XB) * HP, 0:F1]
                        .rearrange("(a q) f -> q a f", q=HP),
                        h_sb[0:HP, :, :])
                    nc.scalar.dma_start(
                        hp_dram[t0 * HP:(t0 + XB) * HP, F1:2 * F1]
                        .rearrange("(a q) f -> q a f", q=HP),
                        h_sb[HP:P, :, :])

            # big tables load behind phase A's writes, overlapping AG_h
            nc.scalar.dma_start(idx16_sb[:], idx16_ext[:])
            nc.scalar.dma_start(val_sb[:], val_ext[:])
            nc.scalar.dma_start(dst_sb[:], dst_ext[:])

            # ---- Phase B: AllGather paired h ----
            nc.gpsimd.collective_compute(
                "AllGather", OP.bypass,
                ins=[hp_dram[:]],
                outs=[HP_full[:]],
                replica_groups=[list(range(N_CORES))])

            # ---- Phase C: z = relu(segsum L1); y2 = z @ W2 + b2 ----
            with tc.tile_pool(name="m1", bufs=2) as m1, \
                 tc.tile_pool(name="s1", bufs=4) as s1, \
                 tc.tile_pool(name="o1", bufs=2) as o1, \
                 tc.tile_pool(name="y1", bufs=2) as y1, \
                 tc.tile_pool(name="psZ", bufs=2,
                              space=bass.MemorySpace.PSUM) as psZ, \
                 tc.tile_pool(name="psT", bufs=2,
                              space=bass.MemorySpace.PSUM) as psT, \
                 tc.tile_pool(name="psY", bufs=2,
                              space=bass.MemorySpace.PSUM) as psY:
                for g in range(n_groups):
                    base = int(grp_base[g])
                    Lg = int(grp_base[g + 1]) - base
                    gtiles = list(range(g * group_tiles,
                                        min((g + 1) * group_tiles, n_tiles)))
                    m_sb = m1.tile([P, Lg, 2 * F1], bf16)
                    for b in range(nb):
                        sA = int(seg[g, b, 0])
                        LA = int(seg[g, b, 1])
                        # chunk: >8 blocks (1024 idxs) per dma_gather hangs
                        # the device (empirically bisected: 8 OK, 9+ hangs)
                        for s0 in range(sA, sA + LA, 8):
                            L = min(8, sA + LA - s0)
                            nc.gpsimd.dma_gather(
                                m_sb[:, s0 - base:s0 - base + L, :],
                                HP_full[b * bank:(b + 1) * bank, :],
                                idx16_sb[:, s0 * 8:(s0 + L) * 8],
                                L * P, L * P, 2 * F1)
                    y2_sb = y1.tile([P, len(gtiles), F2], bf16)
                    for ti, t in enumerate(gtiles):
                        cl = tile_cols[t]
                        z_ps = psZ.tile([P, F1], fp32)
                        nmm = 2 * len(cl)
                        k = 0
                        for j in cl:
                            s_sb = s1.tile([P, 2 * P], bf16)
                            nc.vector.tensor_scalar(
                                out=s_sb[:], in0=iotab[:],
                                scalar1=dst_sb[:, j:j + 1],
                                scalar2=val_sb[:, j:j + 1],
                                op0=OP.is_equal, op1=OP.mult)
                            nc.tensor.matmul(z_ps[:], s_sb[:, 0:P],
                                             m_sb[:, j - base, 0:F1],
                                             start=(k == 0),
                                             stop=(k == nmm - 1))
                            k += 1
                            nc.tensor.matmul(z_ps[:], s_sb[:, P:2 * P],
                                             m_sb[:, j - base, F1:2 * F1],
                                             start=(k == 0),
                                             stop=(k == nmm - 1))
                            k += 1
                        z_sb = o1.tile([P, F1], fp32)
                        nc.scalar.activation(z_sb[:], z_ps[:], AF.Relu)
                        zT_ps = psT.tile([F1, P], fp32)
                        nc.tensor.transpose(zT_ps[:], z_sb[:], ident[:])
                        zT_sb = o1.tile([F1, P], fp32)
                        nc.scalar.activation(zT_sb[:], zT_ps[:], AF.Copy)
                        y2_ps = psY.tile([P, F2], fp32)
                        nc.tensor.matmul(y2_ps[:], ones_sb[:], b2_sb[:],
                                         start=True, stop=False)
                        nc.tensor.matmul(y2_ps[:], zT_sb[:], w2_sb[:],
                                         start=False, stop=True)
                        nc.scalar.activation(y2_sb[:, ti, :], y2_ps[:],
                                             AF.Copy)
                    # quad store of the group's y2 tiles
                    t0 = gtiles[0]
                    nt = len(gtiles)
                    for q in range(4):
                        nc.scalar.dma_start(
                            y2q_dram[t0 * QP:(t0 + nt) * QP,
                                     q * F2:(q + 1) * F2]
                            .rearrange("(a r) f -> r a f", r=QP),
                            y2_sb[q * QP:(q + 1) * QP, :, :])

            # L2 tables load behind phase C's stores, overlapping AG_y2
            nc.scalar.dma_start(idx2_sb[:], idx2_ext[:])
            nc.scalar.dma_start(val2_sb[:], val2_ext[:])
            nc.scalar.dma_start(dst3_sb[:], dst3_ext[:])
            nc.scalar.dma_start(iota512[:], iota512_ext[:])

            # ---- Phase D: AllGather quad y2 ----
            nc.gpsimd.collective_compute(
                "AllGather", OP.bypass,
                ins=[y2q_dram[:]],
                outs=[Y2Q_full[:]],
                replica_groups=[list(range(N_CORES))])

            # ---- Phase E: out = segsum L2 (quad, single bank) ----
            with tc.tile_pool(name="m2", bufs=2) as m2, \
                 tc.tile_pool(name="s2", bufs=4) as s2, \
                 tc.tile_pool(name="o2", bufs=2) as o2, \
                 tc.tile_pool(name="psO", bufs=2,
                              space=bass.MemorySpace.PSUM) as psO:
                for g in range(layout2["n_groups"]):
                    base = int(grp_base2[g])
                    Lg = int(grp_base2[g + 1]) - base
                    m_sb = m2.tile([P, Lg, 4 * F2], bf16)
                    for s0 in range(base, base + Lg, 8):
                        L = min(8, base + Lg - s0)
                        nc.gpsimd.dma_gather(
                            m_sb[:, s0 - base:s0 - base + L, :],
                            Y2Q_full[:],
                            idx2_sb[:, s0 * 8:(s0 + L) * 8],
                            L * P, L * P, 4 * F2)
                    for t in range(g * group_tiles,
                                   min((g + 1) * group_tiles, n_tiles)):
                        cl = tile_cols2[t]
                        o_ps = psO.tile([P, F2], fp32)
                        nmm = 4 * len(cl)
                        k = 0
                        for j in cl:
                            s_sb = s2.tile([P, 4 * P], bf16)
                            nc.vector.tensor_scalar(
                                out=s_sb[:], in0=iota512[:],
                                scalar1=dst3_sb[:, j:j + 1],
                                scalar2=val2_sb[:, j:j + 1],
                                op0=OP.is_equal, op1=OP.mult)
                            for q in range(4):
                                nc.tensor.matmul(
                                    o_ps[:], s_sb[:, q * P:(q + 1) * P],
                                    m_sb[:, j - base, q * F2:(q + 1) * F2],
                                    start=(k == 0),
                                    stop=(k == nmm - 1))
                                k += 1
                        o_sb = o2.tile([P, F2], fp32)
                        nc.scalar.activation(o_sb[:], o_ps[:], AF.Copy)
                        nc.scalar.dma_start(
                            out_ext[t * P:(t + 1) * P, :], o_sb[:])

    nc.compile()
    return nc


def _run(rows, cols, vals, x, W1, b1, W2, b2, group_tiles=4,
         trace=False, full_results=False):
    from concourse.bass_utils import run_bass_kernel_spmd

    n_nodes, F1 = x.shape
    F2 = W2.shape[1]
    NP_ = -(-n_nodes // (N_CORES * P)) * (N_CORES * P)
    per_core = NP_ // N_CORES
    n_tiles = per_core // P

    PR = NP_ // 2
    nb = -(-PR // 32767)
    while PR % nb:
        nb += 1
    bank_rows = PR // nb
    assert bank_rows <= 32767

    idx16, val_a, dst_a, layout = _preprocess(
        rows, cols, vals, per_core, n_tiles, group_tiles, bank_rows, nb)
    idx16_2, val2_a, dst3_a, layout2 = _preprocess2(
        rows, cols, vals, per_core, n_tiles, group_tiles)

    import ml_dtypes
    bf = ml_dtypes.bfloat16
    x_pad = np.zeros((NP_, F1), dtype=np.float32)
    x_pad[:n_nodes] = x
    b1r = np.ascontiguousarray(b1.reshape(1, F1).astype(bf))
    b2r = np.ascontiguousarray(b2.reshape(1, F2).astype(np.float32))

    nc = _build_program(F1, F2, per_core, n_tiles, group_tiles, bank_rows, nb,
                        layout, layout2)

    in_maps = []
    for c in range(N_CORES):
        in_maps.append({
            "xT": np.ascontiguousarray(
                x_pad[c * per_core:(c + 1) * per_core].T.astype(bf)),
            "w1": np.ascontiguousarray(W1.astype(bf)),
            "b1": b1r,
            "w2": np.ascontiguousarray(W2.astype(np.float32)),
            "b2": b2r,
            "idx16": np.ascontiguousarray(idx16[c]),
            "val": np.ascontiguousarray(val_a[c]),
            "dst": np.ascontiguousarray(dst_a[c]),
            "iotab": np.tile(np.arange(2 * P, dtype=bf), (P, 1)),
            "idx2": np.ascontiguousarray(idx16_2[c]),
            "val2": np.ascontiguousarray(val2_a[c]),
            "dst3": np.ascontiguousarray(dst3_a[c]),
            "iota512": np.tile(np.arange(4 * P, dtype=np.float16), (P, 1)),
            "ident": np.eye(P, dtype=np.float32),
        })

    import time as _time
    t0 = _time.perf_counter()
    res = run_bass_kernel_spmd(nc, in_maps, core_ids=list(range(N_CORES)),
                               trace=trace)
    wall_ns = int((_time.perf_counter() - t0) * 1e9)
    t_ns = res.exec_time_ns if res.exec_time_ns is not None else wall_ns
    out = np.concatenate([res.results[c]["out"] for c in range(N_CORES)],
                         axis=0)[:n_nodes]
    if full_results:
        return out, res, (idx16, val_a, dst_a, layout, per_core,
                          n_tiles, bank_rows, nb)
    return out, t_ns


def kernel(**inputs):
    global last_exec_time_ns
    trace = os.environ.get("KERNEL_TRACE", "0") == "1"
    out, t_ns = _run(inputs["rows"], inputs["cols"], inputs["vals"],
                     inputs["x"], inputs["W1"], inputs["b1"],
                     inputs["W2"], inputs["b2"], trace=trace)
    last_exec_time_ns = t_ns
    return out
